# revision 19
# baseline (speedup 1.0000x reference)
"""Trainium2 Bass kernel for nn_Conv_block_57690000720236.

Reference computation (per batch image b):
  - 3x3 SAME conv "high" branch: 64ch -> 64ch
  - low branch: 3x3 conv 64ch -> 16ch, then 1x1 conv 16ch -> 64ch
  - output position (b,y,x) takes the high value if its flat index is in
    mask_idx, the low value if in inv_mask_idx (inv wins on overlap), 0 if
    in neither.

Strategy (8 NeuronCores, data-parallel over batch; core b does image b):
  - The low branch is folded on the host (W_low = w2 @ w1), and the weight
    blob's output columns 64:128 hold W_low.  One dense set of 3x3-conv
    matmuls per 512-position chunk then yields psum[0:64] = high,
    psum[64:128] = low.
  - Routing:  out = select(mask, low, high).  Weight columns 64:128 hold
    W_low, so one ScalarE copy evicts the whole psum chunk to SBUF f16
    (high lands in the output tile rows 0:64, low in scratch rows
    64:128), an SBUF->SBUF DMA moves low to partitions 0:64, and one
    all-SBUF DVE copy_predicated applies the u8 mask -- the PE only runs
    the 6 conv matmuls per chunk.
  - Layout: channels on SBUF partitions.  Partitions 0:64 hold 18 rows of
    the zero-padded image (130 cols); partitions 64:128 the same shifted
    down one row (host-staged).  Per chunk: 3 K=128 matmuls contract the
    (ky=0,ky=1) tap pairs; the 3 ky=2 taps are also issued as full K=128
    matmuls with zero weights in the unused row
    half: half-array (K=64) matmuls make the PE activity monitor (HAM)
    re-throttle the clock to 1.2GHz, while an all-K=128 stream runs warm
    at 2.4GHz (~2x).  Image/mask tiles are DMA-loaded just-in-time, two
    tiles ahead of compute.
"""

import numpy as np

import concourse.bacc as bacc
import concourse.mybir as mybir
import concourse.tile as tile
from concourse.bass_utils import run_bass_kernel_spmd

B, CIN, H, W = 8, 64, 128, 128
COUT, KER = 64, 3
NPOS = H * W                 # 16384 positions per core
WP = W + 2                   # padded row length 130
N_TILES = 8                  # image row-tiles held in SBUF
TILE_OUT_ROWS = H // N_TILES     # 16 output rows per tile
TILE_P_ROWS = TILE_OUT_ROWS + 2  # 18 padded rows held per tile
CHUNK_ROWS = 4               # output rows per matmul chunk
CHUNK = CHUNK_ROWS * W       # 512 positions per chunk
CHUNKS_PER_TILE = TILE_OUT_ROWS // CHUNK_ROWS
TILE_POS = TILE_OUT_ROWS * W     # 2048 positions per tile
NCHUNKS = N_TILES * CHUNKS_PER_TILE  # 32
F32 = mybir.dt.float32
F16 = mybir.dt.float16
U8 = mybir.dt.uint8
WCOLS = 6 * 128              # weight blob columns


def _build_program(need_zero_fix: bool):
    nc = bacc.Bacc("TRN2", target_bir_lowering=False, debug=False, num_devices=B)

    inx_d = nc.dram_tensor(
        "inxs", [N_TILES, 128, TILE_P_ROWS * WP], F16, kind="ExternalInput"
    )
    w_d = nc.dram_tensor("wblob", [128, WCOLS], F16, kind="ExternalInput")
    m_d = nc.dram_tensor("mlow", [COUT, NPOS], U8, kind="ExternalInput")
    if need_zero_fix:
        mz_d = nc.dram_tensor("mzero", [COUT, NPOS], U8, kind="ExternalInput")
    out_d = nc.dram_tensor("out", [COUT, NPOS], F16, kind="ExternalOutput")

    with tile.TileContext(nc) as tc:
        with (
            tc.tile_pool(name="const", bufs=1) as cpool,
            tc.tile_pool(name="img", bufs=1) as ipool,
            tc.tile_pool(name="outp", bufs=3) as opool,
            tc.tile_pool(name="psum", bufs=8, space="PSUM") as pspool,
        ):
            wt = cpool.tile([128, WCOLS], F16, tag="wblob")
            # split the weight load so the first pair blocks land ASAP
            nc.gpsimd.dma_start(wt[:, 0:384], w_d[:, 0:384])
            nc.gpsimd.dma_start(wt[:, 384:WCOLS], w_d[:, 384:WCOLS])
            mt = cpool.tile([COUT, NPOS], U8, tag="mlow")
            mvbuf = cpool.tile([COUT, 4 * CHUNK], F16, tag="mvbuf")
            if need_zero_fix:
                mzt = cpool.tile([COUT, NPOS], U8, tag="mzero")
                zt = cpool.tile([COUT, CHUNK], F32, tag="zeros")
                nc.any.memset(zt[:], 0.0)

            # Image tiles are loaded just-in-time (two tiles ahead) so the
            # first chunk's matmuls are not stuck behind a long DMA issue
            # queue at kernel start; same for the per-tile mask slices.
            imgs = [
                ipool.tile([128, TILE_P_ROWS * WP], F16, tag=f"img{i}",
                           name=f"img{i}")
                for i in range(N_TILES)
            ]

            def load_tile(i):
                nc.sync.dma_start(imgs[i][:], inx_d[i, :, :])
                nc.gpsimd.dma_start(
                    mt[:, i * TILE_POS:(i + 1) * TILE_POS],
                    m_d[:, i * TILE_POS:(i + 1) * TILE_POS],
                )
                if need_zero_fix:
                    nc.gpsimd.dma_start(
                        mzt[:, i * TILE_POS:(i + 1) * TILE_POS],
                        mz_d[:, i * TILE_POS:(i + 1) * TILE_POS],
                    )

            # first image tile split across four DMA queues (one queue
            # moves ~150GB/s, so a single 600KB transfer would gate the
            # first matmul by ~4us); chunk 0's rows land first.
            nc.sync.dma_start(imgs[0][0:64, 0:8 * WP], inx_d[0, 0:64, 0:8 * WP])
            nc.scalar.dma_start(
                imgs[0][64:128, 0:8 * WP], inx_d[0, 64:128, 0:8 * WP]
            )
            nc.sync.dma_start(
                imgs[0][0:64, 8 * WP:TILE_P_ROWS * WP],
                inx_d[0, 0:64, 8 * WP:TILE_P_ROWS * WP],
            )
            nc.scalar.dma_start(
                imgs[0][64:128, 8 * WP:TILE_P_ROWS * WP],
                inx_d[0, 64:128, 8 * WP:TILE_P_ROWS * WP],
            )
            nc.gpsimd.dma_start(mt[:, 0:TILE_POS], m_d[:, 0:TILE_POS])
            if need_zero_fix:
                nc.gpsimd.dma_start(mzt[:, 0:TILE_POS], mz_d[:, 0:TILE_POS])
            load_tile(1)

            pts = [None] * NCHUNKS   # psum accumulators
            osb = [None] * N_TILES   # per-tile output staging

            def finish(k):
                # merge chunk k: overwrite high with low where mask is set
                ik, jk = divmod(k, CHUNKS_PER_TILE)
                so = jk * CHUNK
                sk = (k % 4) * CHUNK
                s = k * CHUNK
                for h in (0, 1):
                    nc.vector.copy_predicated(
                        osb[ik][0:64, so + h * 256:so + (h + 1) * 256],
                        mt[:, s + h * 256:s + (h + 1) * 256],
                        mvbuf[:, sk + h * 256:sk + (h + 1) * 256],
                    )
                if need_zero_fix:
                    nc.vector.copy_predicated(
                        osb[ik][0:64, so:so + CHUNK], mzt[:, s:s + CHUNK], zt[:]
                    )
                if ik == N_TILES - 1:
                    nc.scalar.dma_start(
                        out_d[:, s:s + CHUNK], osb[ik][0:64, so:so + CHUNK]
                    )
                elif jk == CHUNKS_PER_TILE - 1:
                    nc.scalar.dma_start(
                        out_d[:, ik * TILE_POS:(ik + 1) * TILE_POS],
                        osb[ik][0:64, :],
                    )

            for j in range(NCHUNKS):
                i, jj = divmod(j, CHUNKS_PER_TILE)
                l0 = jj * CHUNK_ROWS
                s = j * CHUNK
                if jj == 0:
                    osb[i] = opool.tile(
                        [128, TILE_POS], F16, tag="osb", name=f"osb{i}"
                    )
                    if i + 2 < N_TILES:
                        load_tile(i + 2)
                v = imgs[i][:].rearrange("p (r x) -> p r x", x=WP)

                pt = pspool.tile([128, CHUNK], F32, tag="acc", name=f"acc{j}")
                pts[j] = pt
                pv = pt[:].rearrange("p (r x) -> p r x", x=W)

                # (ky=0,ky=1) tap pairs: K=128, one per kx
                for c in range(3):
                    nc.tensor.matmul(
                        pv,
                        wt[:, c * 128:(c + 1) * 128],
                        v[:, l0:l0 + CHUNK_ROWS, c:c + W],
                        start=(c == 0),
                        stop=False,
                    )
                # ky=2 taps, one per kx: K=128 with zero weights in
                # rows 64:128 (full-array matmuls keep the PE HAM activity
                # monitor warm at 2.4GHz; K=64 would re-throttle to 1.2)
                for c in range(3):
                    nc.tensor.matmul(
                        pv,
                        wt[:, (3 + c) * 128:(4 + c) * 128],
                        v[:, l0 + 2:l0 + 2 + CHUNK_ROWS, c:c + W],
                        start=False,
                        stop=(c == 2),
                    )
                # one ScalarE copy evicts the whole psum chunk: high into
                # the output rows, low into the scratch rows; an SBUF->SBUF
                # DMA then moves low to partitions 0:64 for the lagged
                # copy_predicated merge.
                sj = (j % 4) * CHUNK
                so_j = jj * CHUNK
                nc.scalar.copy(osb[i][:, so_j:so_j + CHUNK], pt[:, :])
                for h in (0, 1):
                    nc.gpsimd.dma_start(
                        mvbuf[:, sj + h * 256:sj + (h + 1) * 256],
                        osb[i][64:128, so_j + h * 256:so_j + (h + 1) * 256],
                    )
                if j > 0:
                    finish(j - 1)

            finish(NCHUNKS - 1)

    nc.compile()
    return nc


def _prepare_host(inx, mask_idx, inv_mask_idx, high_w, low1_w, low2_w):
    inx = np.asarray(inx, dtype=np.float32)
    mask_idx = np.asarray(mask_idx).astype(np.int64)
    inv_mask_idx = np.asarray(inv_mask_idx).astype(np.int64)
    high_w = np.asarray(high_w, dtype=np.float32)
    low1_w = np.asarray(low1_w, dtype=np.float32)
    low2_w = np.asarray(low2_w, dtype=np.float32)

    # zero-padded images P [B, 64, 130, 130]
    inxp = np.zeros((B, CIN, H + 2, WP), np.float32)
    inxp[:, :, 1:-1, 1:-1] = inx
    # staged layout: tile i partitions 0:64 = P rows 16i..16i+17,
    # partitions 64:128 = the same shifted down one row
    stage = np.zeros((B, N_TILES, 128, TILE_P_ROWS, WP), np.float16)
    for i in range(N_TILES):
        tp = i * TILE_OUT_ROWS
        stage[:, i, 0:64] = inxp[:, :, tp:tp + TILE_P_ROWS]
        nb = min(TILE_P_ROWS, (H + 2) - (tp + 1))
        stage[:, i, 64:128, :nb] = inxp[:, :, tp + 1:tp + 1 + nb]
    stage = stage.reshape(B, N_TILES, 128, TILE_P_ROWS * WP)

    # fold the low branch: W_low[o, c, ky, kx] = sum_m w2[o, m] w1[m, c, ky, kx]
    w2 = low2_w.reshape(COUT, -1).astype(np.float64)
    wl = np.einsum("om,mckl->ockl", w2, low1_w.astype(np.float64))
    wd = wl.astype(np.float32)  # low-branch weights (output cols 64:128)
    wh = high_w

    # weight blob [128, 768] f16; lhsT[k, m]: k = input channel row, m = out col
    #   cols c*128..c*128+127 (c in 0,1,2): K=128 (ky=0, ky=1) pair for kx=c
    #   cols (3+c)*128..: rows 0:64 = (ky=2, kx=c) as [wh | wd], rows 64:128 = 0
    blob = np.zeros((128, WCOLS), np.float16)
    for c in range(3):
        blk = blob[:, c * 128:(c + 1) * 128]
        blk[0:64, 0:64] = wh[:, :, 0, c].T
        blk[0:64, 64:128] = wd[:, :, 0, c].T
        blk[64:128, 0:64] = wh[:, :, 1, c].T
        blk[64:128, 64:128] = wd[:, :, 1, c].T
        sblk = blob[:, (3 + c) * 128:(4 + c) * 128]
        sblk[0:64, 0:64] = wh[:, :, 2, c].T
        sblk[0:64, 64:128] = wd[:, :, 2, c].T

    ntotal = B * NPOS
    in_mask = np.zeros(ntotal, dtype=bool)
    in_inv = np.zeros(ntotal, dtype=bool)
    in_mask[mask_idx] = True
    in_inv[inv_mask_idx] = True
    neither = ~(in_mask | in_inv)
    need_zero_fix = bool(neither.any())

    in_maps = []
    for b in range(B):
        sl = slice(b * NPOS, (b + 1) * NPOS)
        mlow = np.ascontiguousarray(
            np.broadcast_to(in_inv[sl].astype(np.uint8)[None, :], (COUT, NPOS))
        )
        m = {"inxs": stage[b], "wblob": blob, "mlow": mlow}
        if need_zero_fix:
            m["mzero"] = np.ascontiguousarray(
                np.broadcast_to(neither[sl].astype(np.uint8)[None, :], (COUT, NPOS))
            )
        in_maps.append(m)
    return in_maps, need_zero_fix


def _run(inputs: dict, trace: bool = False):
    in_maps, need_zero_fix = _prepare_host(**inputs)
    nc = _build_program(need_zero_fix)
    res = run_bass_kernel_spmd(nc, in_maps, list(range(B)), trace=trace)
    out = np.stack(
        [res.results[b]["out"].reshape(COUT, H, W) for b in range(B)]
    ).astype(np.float32)
    return out, res


def kernel(**inputs) -> np.ndarray:
    out, _ = _run(inputs, trace=False)
    return out


# revision 21
# speedup vs baseline: 1.1595x; 1.1595x over previous
"""Trainium2 Bass kernel for nn_Conv_block_57690000720236.

Reference computation (per batch image b):
  - 3x3 SAME conv "high" branch: 64ch -> 64ch
  - low branch: 3x3 conv 64ch -> 16ch, then 1x1 conv 16ch -> 64ch
  - output position (b,y,x) takes the high value if its flat index is in
    mask_idx, the low value if in inv_mask_idx (inv wins on overlap), 0 if
    in neither.

Strategy (8 NeuronCores, data-parallel over batch; core b does image b):
  - The low branch is folded on the host (W_low = w2 @ w1), and the weight
    blob's output columns 64:128 hold W_low.  One dense set of 3x3-conv
    matmuls per 512-position chunk then yields psum[0:64] = high,
    psum[64:128] = low.
  - Routing:  out = select(mask, low, high).  Weight columns 64:128 hold
    W_low, so one ScalarE copy evicts the whole psum chunk to SBUF f16
    (high lands in the output tile rows 0:64, low in scratch rows
    64:128), an SBUF->SBUF DMA moves low to partitions 0:64, and one
    all-SBUF DVE copy_predicated applies the u8 mask -- the PE only runs
    the 6 conv matmuls per chunk.
  - Layout: channels on SBUF partitions.  Partitions 0:64 hold 18 rows of
    the zero-padded image (130 cols); partitions 64:128 the same shifted
    down one row (host-staged).  Per chunk: 3 K=128 matmuls contract the
    (ky=0,ky=1) tap pairs; the 3 ky=2 taps are also issued as full K=128
    matmuls with zero weights in the unused row
    half: half-array (K=64) matmuls make the PE activity monitor (HAM)
    re-throttle the clock to 1.2GHz, while an all-K=128 stream runs warm
    at 2.4GHz (~2x).  Image/mask tiles are DMA-loaded just-in-time, two
    tiles ahead of compute.
"""

import numpy as np

import concourse.bacc as bacc
import concourse.mybir as mybir
import concourse.tile as tile
from concourse.bass_utils import run_bass_kernel_spmd

B, CIN, H, W = 8, 64, 128, 128
COUT, KER = 64, 3
NPOS = H * W                 # 16384 positions per core
WP = W + 2                   # padded row length 130
N_TILES = 8                  # image row-tiles held in SBUF
TILE_OUT_ROWS = H // N_TILES     # 16 output rows per tile
TILE_P_ROWS = TILE_OUT_ROWS + 2  # 18 padded rows held per tile
CHUNK_ROWS = 4               # output rows per matmul chunk
CHUNK = CHUNK_ROWS * W       # 512 positions per chunk
CHUNKS_PER_TILE = TILE_OUT_ROWS // CHUNK_ROWS
TILE_POS = TILE_OUT_ROWS * W     # 2048 positions per tile
NCHUNKS = N_TILES * CHUNKS_PER_TILE  # 32
F32 = mybir.dt.float32
F16 = mybir.dt.float16
U8 = mybir.dt.uint8
WCOLS = 6 * 128              # weight blob columns


def _build_program(need_zero_fix: bool):
    nc = bacc.Bacc("TRN2", target_bir_lowering=False, debug=False, num_devices=B)

    inx_d = nc.dram_tensor(
        "inxs", [N_TILES, 128, TILE_P_ROWS * WP], F16, kind="ExternalInput"
    )
    w_d = nc.dram_tensor("wblob", [128, WCOLS], F16, kind="ExternalInput")
    m_d = nc.dram_tensor("mlow", [COUT, NPOS], U8, kind="ExternalInput")
    if need_zero_fix:
        mz_d = nc.dram_tensor("mzero", [COUT, NPOS], U8, kind="ExternalInput")
    out_d = nc.dram_tensor("out", [COUT, NPOS], F16, kind="ExternalOutput")

    with tile.TileContext(nc) as tc:
        with (
            tc.tile_pool(name="const", bufs=1) as cpool,
            tc.tile_pool(name="img", bufs=1) as ipool,
            tc.tile_pool(name="outp", bufs=3) as opool,
            tc.tile_pool(name="psum", bufs=8, space="PSUM") as pspool,
        ):
            wt = cpool.tile([128, WCOLS], F16, tag="wblob")
            # split the weight load so the first pair blocks land ASAP
            nc.gpsimd.dma_start(wt[:, 0:384], w_d[:, 0:384])
            nc.gpsimd.dma_start(wt[:, 384:WCOLS], w_d[:, 384:WCOLS])
            mt = cpool.tile([COUT, NPOS], U8, tag="mlow")
            mvbuf = cpool.tile([COUT, 4 * CHUNK], F16, tag="mvbuf")
            if need_zero_fix:
                mzt = cpool.tile([COUT, NPOS], U8, tag="mzero")
                zt = cpool.tile([COUT, CHUNK], F32, tag="zeros")
                nc.any.memset(zt[:], 0.0)

            # Image tiles are loaded just-in-time (two tiles ahead) so the
            # first chunk's matmuls are not stuck behind a long DMA issue
            # queue at kernel start; same for the per-tile mask slices.
            imgs = [
                ipool.tile([128, TILE_P_ROWS * WP], F16, tag=f"img{i}",
                           name=f"img{i}")
                for i in range(N_TILES)
            ]

            def load_tile(i):
                nc.sync.dma_start(imgs[i][:], inx_d[i, :, :])
                nc.gpsimd.dma_start(
                    mt[:, i * TILE_POS:(i + 1) * TILE_POS],
                    m_d[:, i * TILE_POS:(i + 1) * TILE_POS],
                )
                if need_zero_fix:
                    nc.gpsimd.dma_start(
                        mzt[:, i * TILE_POS:(i + 1) * TILE_POS],
                        mz_d[:, i * TILE_POS:(i + 1) * TILE_POS],
                    )

            # first image tile split across four DMA queues (one queue
            # moves ~150GB/s, so a single 600KB transfer would gate the
            # first matmul by ~4us); chunk 0's rows land first.
            nc.sync.dma_start(imgs[0][0:64, 0:8 * WP], inx_d[0, 0:64, 0:8 * WP])
            nc.scalar.dma_start(
                imgs[0][64:128, 0:8 * WP], inx_d[0, 64:128, 0:8 * WP]
            )
            nc.sync.dma_start(
                imgs[0][0:64, 8 * WP:TILE_P_ROWS * WP],
                inx_d[0, 0:64, 8 * WP:TILE_P_ROWS * WP],
            )
            nc.scalar.dma_start(
                imgs[0][64:128, 8 * WP:TILE_P_ROWS * WP],
                inx_d[0, 64:128, 8 * WP:TILE_P_ROWS * WP],
            )
            nc.gpsimd.dma_start(mt[:, 0:TILE_POS], m_d[:, 0:TILE_POS])
            if need_zero_fix:
                nc.gpsimd.dma_start(mzt[:, 0:TILE_POS], mz_d[:, 0:TILE_POS])
            load_tile(1)

            pts = [None] * NCHUNKS   # psum accumulators
            osb = [None] * N_TILES   # per-tile output staging

            def finish(k):
                # merge chunk k: overwrite high with low where mask is set
                ik, jk = divmod(k, CHUNKS_PER_TILE)
                so = jk * CHUNK
                sk = (k % 4) * CHUNK
                s = k * CHUNK
                nc.vector.copy_predicated(
                    osb[ik][0:64, so:so + CHUNK],
                    mt[:, s:s + CHUNK],
                    mvbuf[:, sk:sk + CHUNK],
                )
                if need_zero_fix:
                    nc.vector.copy_predicated(
                        osb[ik][0:64, so:so + CHUNK], mzt[:, s:s + CHUNK], zt[:]
                    )
                # output DMAs issue from the scalar queue: on the sync
                # queue their wait-for-preds would block later image loads
                if ik == N_TILES - 1:
                    nc.scalar.dma_start(
                        out_d[:, s:s + CHUNK], osb[ik][0:64, so:so + CHUNK]
                    )
                elif jk == CHUNKS_PER_TILE - 1:
                    nc.scalar.dma_start(
                        out_d[:, ik * TILE_POS:(ik + 1) * TILE_POS],
                        osb[ik][0:64, :],
                    )

            for j in range(NCHUNKS):
                i, jj = divmod(j, CHUNKS_PER_TILE)
                l0 = jj * CHUNK_ROWS
                s = j * CHUNK
                if jj == 0:
                    osb[i] = opool.tile(
                        [128, TILE_POS], F16, tag="osb", name=f"osb{i}"
                    )
                    if i + 2 < N_TILES:
                        load_tile(i + 2)
                v = imgs[i][:].rearrange("p (r x) -> p r x", x=WP)

                pt = pspool.tile([128, CHUNK], F32, tag="acc", name=f"acc{j}")
                pts[j] = pt
                pv = pt[:].rearrange("p (r x) -> p r x", x=W)

                # (ky=0,ky=1) tap pairs: K=128, one per kx
                for c in range(3):
                    nc.tensor.matmul(
                        pv,
                        wt[:, c * 128:(c + 1) * 128],
                        v[:, l0:l0 + CHUNK_ROWS, c:c + W],
                        start=(c == 0),
                        stop=False,
                    )
                # ky=2 taps, one per kx: K=128 with zero weights in
                # rows 64:128 (full-array matmuls keep the PE HAM activity
                # monitor warm at 2.4GHz; K=64 would re-throttle to 1.2)
                for c in range(3):
                    nc.tensor.matmul(
                        pv,
                        wt[:, (3 + c) * 128:(4 + c) * 128],
                        v[:, l0 + 2:l0 + 2 + CHUNK_ROWS, c:c + W],
                        start=False,
                        stop=(c == 2),
                    )
                # one ScalarE copy evicts the whole psum chunk: high into
                # the output rows, low into the scratch rows; an SBUF->SBUF
                # DMA then moves low to partitions 0:64 for the lagged
                # copy_predicated merge.
                sj = (j % 4) * CHUNK
                so_j = jj * CHUNK
                nc.scalar.copy(osb[i][:, so_j:so_j + CHUNK], pt[:, :])
                nc.gpsimd.dma_start(
                    mvbuf[:, sj:sj + CHUNK],
                    osb[i][64:128, so_j:so_j + CHUNK],
                )
                if j > 0:
                    finish(j - 1)

            finish(NCHUNKS - 1)

    nc.compile()
    return nc


def _prepare_host(inx, mask_idx, inv_mask_idx, high_w, low1_w, low2_w):
    inx = np.asarray(inx, dtype=np.float32)
    mask_idx = np.asarray(mask_idx).astype(np.int64)
    inv_mask_idx = np.asarray(inv_mask_idx).astype(np.int64)
    high_w = np.asarray(high_w, dtype=np.float32)
    low1_w = np.asarray(low1_w, dtype=np.float32)
    low2_w = np.asarray(low2_w, dtype=np.float32)

    # zero-padded images P [B, 64, 130, 130]
    inxp = np.zeros((B, CIN, H + 2, WP), np.float32)
    inxp[:, :, 1:-1, 1:-1] = inx
    # staged layout: tile i partitions 0:64 = P rows 16i..16i+17,
    # partitions 64:128 = the same shifted down one row
    stage = np.zeros((B, N_TILES, 128, TILE_P_ROWS, WP), np.float16)
    for i in range(N_TILES):
        tp = i * TILE_OUT_ROWS
        stage[:, i, 0:64] = inxp[:, :, tp:tp + TILE_P_ROWS]
        nb = min(TILE_P_ROWS, (H + 2) - (tp + 1))
        stage[:, i, 64:128, :nb] = inxp[:, :, tp + 1:tp + 1 + nb]
    stage = stage.reshape(B, N_TILES, 128, TILE_P_ROWS * WP)

    # fold the low branch: W_low[o, c, ky, kx] = sum_m w2[o, m] w1[m, c, ky, kx]
    w2 = low2_w.reshape(COUT, -1).astype(np.float64)
    wl = np.einsum("om,mckl->ockl", w2, low1_w.astype(np.float64))
    wd = wl.astype(np.float32)  # low-branch weights (output cols 64:128)
    wh = high_w

    # weight blob [128, 768] f16; lhsT[k, m]: k = input channel row, m = out col
    #   cols c*128..c*128+127 (c in 0,1,2): K=128 (ky=0, ky=1) pair for kx=c
    #   cols (3+c)*128..: rows 0:64 = (ky=2, kx=c) as [wh | wd], rows 64:128 = 0
    blob = np.zeros((128, WCOLS), np.float16)
    for c in range(3):
        blk = blob[:, c * 128:(c + 1) * 128]
        blk[0:64, 0:64] = wh[:, :, 0, c].T
        blk[0:64, 64:128] = wd[:, :, 0, c].T
        blk[64:128, 0:64] = wh[:, :, 1, c].T
        blk[64:128, 64:128] = wd[:, :, 1, c].T
        sblk = blob[:, (3 + c) * 128:(4 + c) * 128]
        sblk[0:64, 0:64] = wh[:, :, 2, c].T
        sblk[0:64, 64:128] = wd[:, :, 2, c].T

    ntotal = B * NPOS
    in_mask = np.zeros(ntotal, dtype=bool)
    in_inv = np.zeros(ntotal, dtype=bool)
    in_mask[mask_idx] = True
    in_inv[inv_mask_idx] = True
    neither = ~(in_mask | in_inv)
    need_zero_fix = bool(neither.any())

    in_maps = []
    for b in range(B):
        sl = slice(b * NPOS, (b + 1) * NPOS)
        mlow = np.ascontiguousarray(
            np.broadcast_to(in_inv[sl].astype(np.uint8)[None, :], (COUT, NPOS))
        )
        m = {"inxs": stage[b], "wblob": blob, "mlow": mlow}
        if need_zero_fix:
            m["mzero"] = np.ascontiguousarray(
                np.broadcast_to(neither[sl].astype(np.uint8)[None, :], (COUT, NPOS))
            )
        in_maps.append(m)
    return in_maps, need_zero_fix


def _run(inputs: dict, trace: bool = False):
    in_maps, need_zero_fix = _prepare_host(**inputs)
    nc = _build_program(need_zero_fix)
    res = run_bass_kernel_spmd(nc, in_maps, list(range(B)), trace=trace)
    out = np.stack(
        [res.results[b]["out"].reshape(COUT, H, W) for b in range(B)]
    ).astype(np.float32)
    return out, res


def kernel(**inputs) -> np.ndarray:
    out, _ = _run(inputs, trace=False)
    return out
